# revision 54
# baseline (speedup 1.0000x reference)
"""Trainium2 Bass kernel: segment-softmax attention pooling.

Computes, for fea [N,256], sorted segment index [N] with S segments:
    gate = softmax_per_segment(fea @ Wg + bg)
    out[s] = sum_{i in s} gate_i * (fea_i @ Wm + bm)      -> [S, 256]

Restructuring (vs the naive reference):
  out[s] = (sum_i w_i fea_i) @ Wm + (sum_i w_i) * bm, so the big
  [N,256]x[256,256] matmul collapses to [S,256]x[256,256] after pooling.
  Softmax skips max-subtraction (logits are ~N(0,1); exp is safe in fp32).

Key layout trick: the host streams prod = fea * Wg (elementwise, per
column) instead of fea. Gate logits then reduce to per-node row sums
(one 4x-mode DVE tensor_scalar per 128-node tile), and the division by
Wg is folded into the epilogue weights Wm' = Wm / Wg[:,None] on the
host. All stream/epilogue tensors are fp16 (PE runs 1 cycle/row vs 4
for fp32); accumulations (PSUM, gate sums) stay fp32.

Sharding: 6250 segments per core; 128-segment blocks; each block's
nodes (sorted index => contiguous) are loaded as T 128-node tiles. Per
tile a one-hot A[i,j] = (idx_i==j)*e_i scatters e-weighted rows (plus a
ones column giving gsum) into PSUM via TensorE. idx (localized per
block, f32) is preloaded to SBUF in one merged const DMA.

Normalization happens in the pooled PSUM drain (ACT mul by 1/gsum), so
the drained gsum column is exactly 1 and the +bm term becomes a
constant rank-1 matmul (ones_row x bm) on PE; empty segments produce
NaN rows that the host zeroes. Engine split per block: all 11 gate
reduces on DVE, one-hot builds alternate ~4xDVE / ~7xGpSimd, scatter/
transpose/epilogue matmuls on PE, exp (in 3 chunks, so early one-hots
never wait on the full reduce chain) + drains on ACT.

Scheduling (tuned against the concourse TimelineSim cost model): the
emission software-pipelines gate(b) with scatter(b-1) interleaved at
tile granularity so no in-order sequencer parks on a cross-engine
dependency; per-engine tile pools avoid cross-engine WAW semaphores;
the ptT/out PSUM drains and the out stores batch two blocks per
instruction; stores lag far behind compute so they never park the SP
queue ahead of the block loads. In steady state the DMA engines run at
~99% (the fp16 feature stream is the roofline).
"""

import numpy as np

from concourse import bacc, mybir, tile
from concourse.bass_utils import run_bass_kernel_spmd
from concourse.masks import make_identity

P = 128
D = 256
N_CORES = 8
S_TOTAL = 50_000
N_TOTAL = 500_000
PAD_IDX = 300.0      # local idx for padding rows: never matches iota 0..127

F32 = mybir.dt.float32
FP16 = mybir.dt.float16

N_ACT_REDUCE = 0     # gate reduces offloaded to ACT (rest on DVE)
N_POOL_ONEHOT = 7    # one-hot builds offloaded to GpSimd (rest on DVE)
N_POOL_ONEHOT2 = 6   # ... on odd blocks (fractional balance)
ONEHOT_ALT = True    # interleave DVE/Pool one-hot tile assignment
PAIR_LOAD = False    # batch block loads in pairs
ILEAVE_LEAD = 8      # reduces emitted before first one-hot; None = no interleave
STORE_LAG = 13       # iterations between a block's finalize emission and its store
BM_ADD_ENG = "dve"   # engine for the bm add: "dve" | "pool"
EXP_SPLITS = 3       # exp emitted in this many chunks (early one-hots start sooner)
PTT_DRAIN = "act"    # engine draining ptT psum->sbuf: "act" | "dve"
OUT_DRAIN = "act"    # engine draining out psum->sbuf: "act" | "dve"
POOLED_DRAIN = "act" # engine for the normalize+drain of pooled psum
STORE_Q = "sp"       # which hwdge queue issues out-stores: "sp" | "act"
LOAD_ALT = False     # alternate load DMAs between SP and ACT queues
BUFS = dict(blk=6, gate=16, junk=10, onehot=18, psb=4, ptsb=4, osb=12, scal=8,
            pooledps=2, ptps=2, outps=2)


def build_program(nblk: int, T: int):
    nc = bacc.Bacc("TRN2", target_bir_lowering=False)

    blk_d = nc.declare_dram_parameter("blk", [nblk, T, P, D + 1], FP16, isOutput=False)
    cf32_d = nc.declare_dram_parameter("cf32", [P, nblk * T + 1], F32, isOutput=False)
    cf16_d = nc.declare_dram_parameter("cf16", [P, 3 * D], FP16, isOutput=False)
    out_d = nc.declare_dram_parameter("out", [nblk * P, D], FP16, isOutput=True)

    with tile.TileContext(nc) as tc:
        with (
            tc.tile_pool(name="const", bufs=1) as cpool,
            tc.tile_pool(name="blk", bufs=BUFS["blk"]) as blkpool,
            tc.tile_pool(name="gate", bufs=BUFS["gate"]) as gpool,
            tc.tile_pool(name="junkd", bufs=BUFS["junk"]) as jpool_d,
            tc.tile_pool(name="junka", bufs=3) as jpool_a,
            tc.tile_pool(name="onehotd", bufs=BUFS["onehot"]) as apool_d,
            tc.tile_pool(name="onehotp", bufs=BUFS["onehot"]) as apool_p,
            tc.tile_pool(name="psb", bufs=BUFS["psb"]) as psbpool,
            tc.tile_pool(name="ptsb", bufs=BUFS["ptsb"]) as ptsbpool,
            tc.tile_pool(name="osb", bufs=BUFS["osb"]) as osbpool,
            tc.tile_pool(name="scal", bufs=BUFS["scal"]) as scpool,
            tc.tile_pool(name="pooledps", bufs=BUFS["pooledps"], space="PSUM") as poolps,
            tc.tile_pool(name="ptps", bufs=BUFS["ptps"], space="PSUM") as ptps,
            tc.tile_pool(name="outps", bufs=BUFS["outps"], space="PSUM") as outps,
        ):
            # ---- constants (two merged DMAs, issued from the ACT hwdge
            # queue so the SP queue starts block loads immediately) ----
            cf32 = cpool.tile([P, nblk * T + 1], F32)
            nc.scalar.dma_start(out=cf32[:], in_=cf32_d[:])
            cf16 = cpool.tile([P, 3 * D], FP16)
            nc.scalar.dma_start(out=cf16[:], in_=cf16_d[:])
            idxl = cf32[:, 0 : nblk * T]
            bgb = cf32[:, nblk * T : nblk * T + 1]
            wm0 = cf16[:, 0:D]
            wm1 = cf16[:, D : 2 * D]
            bm_full = cf16[:, 2 * D : 3 * D]

            ones_row = cpool.tile([1, P], FP16)
            nc.gpsimd.memset(ones_row[:], 1.0)
            iota_i = cpool.tile([P, P], mybir.dt.int32)
            nc.gpsimd.iota(iota_i[:], pattern=[[1, P]], base=0, channel_multiplier=0)
            iotaf = cpool.tile([P, P], FP16)
            nc.vector.tensor_copy(out=iotaf[:], in_=iota_i[:])
            ident = cpool.tile([P, P], FP16)
            make_identity(nc, ident[:])

            # Software-pipelined: gate phase of block b is emitted alongside
            # the scatter/epilogue phase of block b-1, so each engine's
            # in-order sequencer rarely parks on a cross-engine dependency.
            # Block loads/stores are batched in pairs (halves DMA overheads).
            state = {}
            loaded = {}

            def load_pair(b):
                w = min(2, nblk - b) if PAIR_LOAD else 1
                blkt2 = blkpool.tile([P, w, T, D + 1], FP16, tag="blk")
                ldq = nc.scalar if (LOAD_ALT and (b // (2 if PAIR_LOAD else 1)) % 2) else nc.sync
                ldq.dma_start(
                    out=blkt2[:], in_=blk_d[b : b + w].rearrange("a t p c -> p a t c")
                )
                for j in range(w):
                    loaded[b + j] = blkt2[:, j]

            def block_pass(b):
                """Emit gate work for block b interleaved per-tile with
                scatter work for block b-1, so DVE alternates
                one-hot(b-1)/reduce(b) and PE gets a steady a_t drip."""
                blkt_b = loaded.pop(b, None)
                prev = state.pop(b - 1, None)
                g = None
                if blkt_b is not None:
                    g = gpool.tile([P, T], F32, tag="g")
                pooled_ps = None
                if prev is not None:
                    pooled_ps = poolps.tile([P, D + 1], F32, tag="pooled")
                    blkt_p, e_p = prev

                def emit_scatter_tile(t):
                    npool = N_POOL_ONEHOT if (b - 1) % 2 == 0 else N_POOL_ONEHOT2
                    if ONEHOT_ALT:
                        # alternate engines so PE consumes from both in parallel
                        ndve = T - npool
                        on_dve = (t % 2 == 1) and (t // 2 < ndve) or (t >= 2 * npool)
                    else:
                        on_dve = t < T - npool
                    a_t = (apool_d if on_dve else apool_p).tile([P, P], FP16, tag="a")
                    eng = nc.vector if on_dve else nc.gpsimd
                    eng.tensor_scalar(
                        out=a_t[:],
                        in0=iotaf[:],
                        scalar1=idxl[:, (b - 1) * T + t : (b - 1) * T + t + 1],
                        scalar2=e_p[:, t : t + 1],
                        op0=mybir.AluOpType.is_equal,
                        op1=mybir.AluOpType.mult,
                    )
                    nc.tensor.matmul(
                        out=pooled_ps[:],
                        lhsT=a_t[:],
                        rhs=blkt_p[:, t, 0 : D + 1],
                        start=(t == 0),
                        stop=(t == T - 1),
                    )

                def emit_reduce_tile(t):
                    on_act = t < N_ACT_REDUCE
                    junk = (jpool_a if on_act else jpool_d).tile([P, D], FP16, tag="junk")
                    if on_act:
                        nc.scalar.activation(
                            out=junk[:],
                            in_=blkt_b[:, t, 0:D],
                            func=mybir.ActivationFunctionType.Copy,
                            accum_out=g[:, t : t + 1],
                        )
                    else:
                        nc.vector.tensor_scalar(
                            out=junk[:],
                            in0=blkt_b[:, t, 0:D],
                            scalar1=1.0,
                            scalar2=None,
                            op0=mybir.AluOpType.mult,
                            op1=mybir.AluOpType.add,
                            accum_out=g[:, t : t + 1],
                        )

                # build the emission schedule for this pass
                sched = []
                if ILEAVE_LEAD is None:
                    if blkt_b is not None:
                        sched += [("r", t) for t in range(T)]
                    if prev is not None:
                        sched += [("s", t) for t in range(T)]
                else:
                    rq = [("r", t) for t in range(T)] if blkt_b is not None else []
                    sq = [("s", t) for t in range(T)] if prev is not None else []
                    sched += rq[:ILEAVE_LEAD]
                    rq = rq[ILEAVE_LEAD:]
                    while rq or sq:
                        if sq:
                            sched.append(sq.pop(0))
                        if rq:
                            sched.append(rq.pop(0))
                for kind, t in sched:
                    if kind == "r":
                        emit_reduce_tile(t)
                    else:
                        emit_scatter_tile(t)

                def emit_exp():
                    if blkt_b is None:
                        return
                    # e = exp(g + bg), in EXP_SPLITS chunks so the first
                    # one-hots of the next pass start before all reduces land
                    e = gpool.tile([P, T], F32, tag="e")
                    bounds_e = [round(i * T / EXP_SPLITS) for i in range(EXP_SPLITS + 1)]
                    for lo, hi in zip(bounds_e[:-1], bounds_e[1:]):
                        if hi > lo:
                            nc.scalar.activation(
                                out=e[:, lo:hi], in_=g[:, lo:hi],
                                func=mybir.ActivationFunctionType.Exp, bias=bgb,
                            )
                    state[b] = (blkt_b, e)

                emit_exp()
                if prev is None:
                    return

                b_p = b - 1
                # scale = 1/gsum (empty segments give inf -> NaN rows; the
                # host zeroes them, so no epsilon is needed)
                scale_t = scpool.tile([P, 1], F32, tag="scale")
                nc.vector.reciprocal(scale_t[:], pooled_ps[:, D : D + 1])

                # drain WITH normalization: pooledN = pooled/gsum; its gsum
                # column becomes exactly 1, so "+bm" is a constant rank-1 on PE
                pooled_sb = psbpool.tile([P, D + 1], FP16, tag="psb")
                if POOLED_DRAIN == "act":
                    nc.scalar.mul(out=pooled_sb[:], in_=pooled_ps[:], mul=scale_t[:])
                else:
                    nc.vector.tensor_scalar_mul(pooled_sb[:], pooled_ps[:], scale_t[:])

                # transpose pooled into the PAIR psum tile (4 transposes per
                # pair land in one tile; drains/epilogue batch per pair to
                # halve the ACT fixed overheads)
                pb = b_p - (b_p % 2)
                j = b_p % 2
                if pb not in pending_pt:
                    w = min(2, nblk - pb)
                    ptT2 = ptps.tile([P, w, D], FP16, tag="pt", name=f"pt{pb}")
                    pending_pt[pb] = ptT2
                ptT2 = pending_pt[pb]
                nc.tensor.transpose(out=ptT2[:, j, 0:P], in_=pooled_sb[:, 0:P], identity=ident[:])
                nc.tensor.transpose(out=ptT2[:, j, P : 2 * P], in_=pooled_sb[:, P : 2 * P], identity=ident[:])

                if j == 1 or b_p == nblk - 1:
                    ptT2 = pending_pt.pop(pb)
                    w = ptT2.shape[1]
                    ptT_sb = ptsbpool.tile([P, w, D], FP16, tag="ptsb", name=f"ptsb{pb}")
                    nc.scalar.copy(out=ptT_sb[:], in_=ptT2[:])

                    # out = pooledN^T.T @ Wm' + 1 x bm  (bm rank-1 is constant
                    # since the normalized gsum column is 1; host zeroes the
                    # NaN rows of empty segments)
                    out_ps = outps.tile([P, w, D], F32, tag="outps", name=f"ops{pb}")
                    for jj in range(w):
                        nc.tensor.matmul(out=out_ps[:, jj], lhsT=ptT_sb[:, jj, 0:P], rhs=wm0, start=True, stop=False)
                        nc.tensor.matmul(out=out_ps[:, jj], lhsT=ptT_sb[:, jj, P : 2 * P], rhs=wm1, start=False, stop=False)
                        nc.tensor.matmul(out=out_ps[:, jj], lhsT=ones_row[:], rhs=bm_full[0:1, :], start=False, stop=True)

                    out2 = osbpool.tile([P, w, D], FP16, tag="osb", name=f"osb{pb}")
                    nc.scalar.copy(out=out2[:], in_=out_ps[:])
                    pending_out[pb] = out2

            def store_pair(pb):
                out2 = pending_out.pop(pb)
                w = out2.shape[1]
                stq = nc.scalar if STORE_Q == "act" else nc.sync
                stq.dma_start(
                    out=out_d[pb * P : (pb + w) * P, :].rearrange(
                        "(a p) c -> p a c", a=w
                    ),
                    in_=out2[:],
                )

            # stores lag far behind compute so the out-store dma_start never
            # parks the in-order SP queue ahead of the next block load (the
            # pipeline's load->finalize latency is several block periods).
            pending_out = {}
            pending_pt = {}
            lstep = 2 if PAIR_LOAD else 1
            for b in range(nblk + 1 + STORE_LAG):
                if b < nblk and b % lstep == 0:
                    load_pair(b)
                block_pass(b)
                pb = b - STORE_LAG
                if pb >= 0 and pb % 2 == 0 and pb in pending_out:
                    store_pair(pb)
            for pb in sorted(pending_out):
                store_pair(pb)

    nc.finalize()
    return nc


def pack_inputs(fea, index, Wg, bg, Wm, bm, n_cores=N_CORES, s_total=S_TOTAL):
    """Host prep: stream prod = fea*Wg (fp16), fold 1/Wg into Wm' = Wm/Wg.

    Per core: nblk 128-segment blocks; block b's nodes live at
    [nlo_b, nlo_b + T*128) in the node stream (overread rows belong to later
    blocks and carry local idx >= 128, so the one-hot zeroes them out).
    """
    fea = np.asarray(fea, dtype=np.float32)
    index = np.asarray(index).astype(np.int64)
    Wg = np.asarray(Wg, dtype=np.float32)
    bg = np.asarray(bg, dtype=np.float32)
    Wm = np.asarray(Wm, dtype=np.float32)
    bm = np.asarray(bm, dtype=np.float32)

    segs_per_core = s_total // n_cores
    nblk = -(-segs_per_core // P)

    seg_lo = []
    for c in range(n_cores):
        base = c * segs_per_core
        for b in range(nblk):
            seg_lo.append(base + min(b * P, segs_per_core))
    seg_lo = np.array(seg_lo)
    bounds = np.searchsorted(index, np.concatenate([seg_lo, [s_total]]), side="left")
    lens = np.diff(bounds)
    T = max(1, int(-(-int(lens.max()) // P)))

    prod = (fea * Wg[:, 0]).astype(np.float16)
    prod_pad = np.concatenate(
        [prod, np.zeros((T * P, D), dtype=np.float16)], axis=0
    )
    index_pad = np.concatenate(
        [index, np.full((T * P,), 10 * s_total, dtype=np.int64)]
    )

    wmp = np.ascontiguousarray(Wm / Wg[:, 0:1]).astype(np.float16)
    bmf = np.ascontiguousarray(np.broadcast_to(bm.reshape(1, D), (P, D))).astype(np.float16)
    bgb = np.full((P, 1), float(bg[0]), dtype=np.float32)

    in_maps = []
    for c in range(n_cores):
        blk = np.empty((nblk, T * P, D + 1), dtype=np.float16)
        blk[:, :, D] = np.float16(1.0)
        idxl = np.empty((nblk, T * P), dtype=np.float32)
        for b in range(nblk):
            i = c * nblk + b
            nlo = int(bounds[i])
            win = slice(nlo, nlo + T * P)
            blk[b, :, 0:D] = prod_pad[win]
            idxl[b] = (index_pad[win] - seg_lo[i]).astype(np.float32)
        # idxl [nblk, T*P] -> [P, nblk*T] (partition-major for one SBUF DMA)
        idxl_sb = idxl.reshape(nblk, T, P).transpose(2, 0, 1).reshape(P, nblk * T)
        cf32 = np.ascontiguousarray(
            np.concatenate([idxl_sb, bgb], axis=1)
        ).astype(np.float32)
        cf16 = np.ascontiguousarray(
            np.concatenate([wmp[0:P], wmp[P : 2 * P], bmf], axis=1)
        ).astype(np.float16)
        in_maps.append(
            {
                "blk": blk.reshape(nblk, T, P, D + 1),
                "cf32": cf32,
                "cf16": cf16,
            }
        )
    return in_maps, nblk, T, segs_per_core


def kernel(fea, Wg, bg, Wm, bm, index):
    in_maps, nblk, T, segs_per_core = pack_inputs(fea, index, Wg, bg, Wm, bm)
    nc = build_program(nblk, T)
    results = run_bass_kernel_spmd(nc, in_maps, list(range(N_CORES))).results
    out = np.empty((S_TOTAL, D), dtype=np.float32)
    for c in range(N_CORES):
        out[c * segs_per_core : (c + 1) * segs_per_core] = (
            results[c]["out"][:segs_per_core].astype(np.float32)
        )
    # device adds bm unconditionally; reference gives 0 for empty segments
    counts = np.bincount(np.asarray(index).astype(np.int64), minlength=S_TOTAL)
    out[counts == 0] = 0.0
    return out
